# revision 1
# baseline (speedup 1.0000x reference)
"""FBPinn forward kernel for Trainium2 (8 NeuronCores, Bass/Tile).

The module computes y(x) = tanh(x) * sum_w [win_w(x)>1e-3] * win_w(x) * MLP_w(x)
for 1M scalar points x in [0,100) -- a fixed 1D function of x. Per core:
  1. evaluate the function at the 4097 knots of a uniform grid over the core's
     12.5-wide domain slice (32768 cells total) using the 30 tiny MLPs
     (block-diagonal-packed PE matmuls, tanh/sigmoid on ACT), masking windows
     exactly at each knot;
  2. assemble per-cell linear records (a0,b0,a1,b1,xsplit) -- two-sided at the
     54 win==1e-3 mask discontinuities so the jumps are reproduced exactly --
     entirely in SBUF, aligned so partition p owns cells [32p, 32p+32);
  3. points are packed (host side) into a (partition, cell)-aligned slot grid:
     cell c -> partition c//32, block c%32, S slots per cell. Interpolation is
     then pure elementwise DVE work with stride-0 broadcast reads of the
     records -- no gather at all.
Host shards points by domain across the 8 cores, packs slots, and un-permutes
the outputs. Piecewise-linear error on this grid is ~2e-6 absmax.
"""

import numpy as np

# ---------------- problem constants (hardcoded from the module spec) ----------
NW = 30
DOM0, DOM1 = 0.0, 100.0
OVERLAP = 0.25
NEURONS = 32
THRESH = 0.001
N = 1_000_000

NCORES = 8
P = 128                      # SBUF partitions
CPP = 24                     # cells per partition
C_LOC = P * CPP              # cells per core (4096)
DW = 12.5                    # per-core domain width
H = DW / C_LOC               # cell width (exact in fp32)
INVH = C_LOC / DW
NG = 3                       # window groups of 4 per core
NSLOT = 4 * NG               # window slots per core
KCHUNK = 512                 # knot columns per pipeline chunk
NKFULL = C_LOC // KCHUNK     # full chunks (6)
KCH_LAST = 128               # short final chunk (last knot + specials + pad)
NKCH = NKFULL + 1
KT = KCHUNK * NKFULL + KCH_LAST
NKNOT = C_LOC + 1            # real grid knots
NB = 16                      # straddle-boundary slots per core
SPEC0 = C_LOC + 8            # first special knot column
S_DEFAULT = 76               # point slots per cell
BIG = np.float32(1e30)


# ---------------- geometry (host, input-independent) --------------------------
def _partition_geom():
    width = (DOM1 - DOM0) / NW
    sub = np.zeros((NW, 2), np.float32)
    for i in range(NW):
        sub[i, 0] = DOM0 if i == 0 else DOM0 + (i - OVERLAP / 2) * width
        sub[i, 1] = DOM1 if i == NW - 1 else DOM0 + (i + 1 + OVERLAP / 2) * width
    means = (sub[:, 0] + sub[:, 1]) / 2
    std = (sub[:, 1] - sub[:, 0]) / 2
    mid = np.zeros(NW + 1, np.float32)
    mid[0] = sub[0, 0]
    mid[-1] = sub[-1, 1]
    for i in range(1, NW):
        mid[i] = (sub[i - 1, 1] + sub[i, 0]) / 2
    return means.astype(np.float32), std.astype(np.float32), mid.astype(np.float32)


def _win64(l, r, x):
    return 1.0 / (1 + np.exp(-(x - l))) / (1 + np.exp(x - r))


def _bisect64(l, r, lo, hi, rising):
    for _ in range(200):
        m = 0.5 * (lo + hi)
        if (_win64(l, r, m) < THRESH) == rising:
            lo = m
        else:
            hi = m
    return 0.5 * (lo + hi)


def _refine_flip_fp32(l32, r32, b64, rising):
    """Exact fp32 x where the reference's jax-fp32 predicate win(x)>1e-3 flips.
    Returns the smallest fp32 x at which the predicate equals its right-side
    state. Falls back to the float64 bisection value if jax is unavailable."""
    try:
        import jax
        import jax.numpy as jnp

        cpu = jax.devices("cpu")[0]
        lo = np.float32(b64 - 5e-5)
        hi = np.float32(b64 + 5e-5)
        xs = np.arange(lo.view(np.int32), hi.view(np.int32) + 1,
                       dtype=np.int32).view(np.float32)
        with jax.default_device(cpu):
            win = np.asarray(
                jax.nn.sigmoid(jnp.asarray(xs) - np.float32(l32))
                * jax.nn.sigmoid(-(jnp.asarray(xs) - np.float32(r32)))
            )
        pred = win > np.float32(THRESH)
        state = pred if rising else ~pred
        if not state.any() or state.all():
            return np.float32(b64)
        k = int(np.argmax(state))
        if not state[k:].all():
            return np.float32(b64)
        return xs[k]
    except Exception:
        return np.float32(b64)


_GEOM = None


def _geometry():
    global _GEOM
    if _GEOM is not None:
        return _GEOM
    means, std, mid = _partition_geom()
    ml = mid[:-1].astype(np.float64)
    mr = mid[1:].astype(np.float64)
    Lb = np.zeros(NW, np.float32)   # window-on lower bound (exact fp32 flip)
    Rb = np.zeros(NW, np.float32)   # window-off upper bound
    for w in range(NW):
        c = 0.5 * (ml[w] + mr[w])
        l64 = _bisect64(ml[w], mr[w], ml[w] - 30, c, rising=True)
        r64 = _bisect64(ml[w], mr[w], c, mr[w] + 30, rising=False)
        Lb[w] = _refine_flip_fp32(mid[w], mid[w + 1], l64, rising=True)
        Rb[w] = _refine_flip_fp32(mid[w], mid[w + 1], r64, rising=False)
    bnds = []
    for w in range(NW):
        if DOM0 < Lb[w] < DOM1:
            bnds.append(float(Lb[w]))
        if DOM0 < Rb[w] < DOM1:
            bnds.append(float(Rb[w]))
    bnds = np.sort(np.array(bnds, np.float64))
    _GEOM = (means, std, mid, Lb, Rb, bnds)
    return _GEOM




def _group_chunk_need():
    """need[ch][g]: does group g contribute anywhere in knot-chunk ch for ANY
    core? Computed from input-independent window geometry."""
    means, std, mid, Lb, Rb, bnds = _geometry()
    need = [[False] * NG for _ in range(NKCH)]
    for core in range(NCORES):
        base = DOM0 + core * DW
        act = [w for w in range(NW) if (Rb[w] > base) and (Lb[w] < base + DW)]
        for slot, w in enumerate(act):
            g = slot // 4
            lo, hi = float(Lb[w]) - base, float(Rb[w]) - base
            for ch in range(NKCH):
                c_lo = ch * KCHUNK * H
                c_hi = (ch + 1) * KCHUNK * H
                if ch == NKCH - 1:
                    c_hi = 1e30    # specials chunk: straddle x anywhere
                    c_lo = 0.0
                if hi > c_lo - 4 * H and lo < c_hi + 4 * H:
                    need[ch][g] = True
    return need

# ---------------- bass program (built once per S, SPMD across 8 cores) --------
_PROGS = {}


def _build_program(S):
    if S in _PROGS:
        return _PROGS[S]
    from concourse import bacc, bass, mybir, tile
    from concourse.bass import IndirectOffsetOnAxis

    f32 = mybir.dt.float32
    f32r = mybir.dt.float32r
    i32 = mybir.dt.int32
    u8 = mybir.dt.uint8
    Act = mybir.ActivationFunctionType
    Op = mybir.AluOpType

    M = CPP * S
    PBLK = 8                      # cell-blocks per point chunk
    PCH = PBLK * S                # point columns per chunk
    NPCH = CPP // PBLK

    nc = bacc.Bacc(None, target_bir_lowering=False)

    x_in = nc.declare_dram_parameter("x_pts", [P, M], f32, isOutput=False)
    base_in = nc.declare_dram_parameter("base_col", [P, 1], f32, isOutput=False)
    c0p_in = nc.declare_dram_parameter("c0p_col", [P, 1], f32, isOutput=False)
    sc1_in = nc.declare_dram_parameter("sc1", [P, NG], f32, isOutput=False)
    bi1_in = nc.declare_dram_parameter("bi1", [P, NG], f32, isOutput=False)
    w2_in = nc.declare_dram_parameter("w2blk", [P, P * NG], f32, isOutput=False)
    w3_in = nc.declare_dram_parameter("w3f", [P, NSLOT * NG], f32, isOutput=False)
    b2_in = nc.declare_dram_parameter("b2c", [P, NG], f32, isOutput=False)
    b3_in = nc.declare_dram_parameter("b3c", [NSLOT, 1], f32, isOutput=False)
    negl_in = nc.declare_dram_parameter("negl", [NSLOT, 1], f32, isOutput=False)
    rr_in = nc.declare_dram_parameter("rr", [NSLOT, 1], f32, isOutput=False)
    lb_in = nc.declare_dram_parameter("lbnd", [NSLOT, 1], f32, isOutput=False)
    rb_in = nc.declare_dram_parameter("rbnd", [NSLOT, 1], f32, isOutput=False)
    knots_in = nc.declare_dram_parameter("knotrep", [P, KT], f32, isOutput=False)
    k12_in = nc.declare_dram_parameter("knot12", [NSLOT, KT], f32, isOutput=False)
    xs_in = nc.declare_dram_parameter("xsplit_col", [P, CPP], f32, isOutput=False)
    itb_in = nc.declare_dram_parameter("invtb_col", [P, CPP], f32, isOutput=False)
    i1tb_in = nc.declare_dram_parameter("inv1mtb_col", [P, CPP], f32, isOutput=False)
    jl_in = nc.declare_dram_parameter("jlist", [NB, 1], i32, isOutput=False)
    wm_in = nc.declare_dram_parameter("wmask", [NSLOT, KT], f32, isOutput=False)
    on_in = nc.declare_dram_parameter("ones12", [NSLOT, 1], f32, isOutput=False)
    y_out = nc.declare_dram_parameter("y_out", [P, M], f32, isOutput=True)

    with tile.TileContext(nc) as tc:
        with (
            tc.tile_pool(name="const", bufs=1) as cpool,
            tc.tile_pool(name="work", bufs=2) as wpool,
            tc.tile_pool(name="pts", bufs=2) as ppool,
            tc.tile_pool(name="psum", bufs=2, space="PSUM") as psum,
            tc.tile_pool(name="dram", bufs=1, space="DRAM") as dpool,
        ):
            v_dram = dpool.tile([KT, 1], f32)            # knot values
            vm_dram = dpool.tile([C_LOC + NB, 1], f32)   # left-limit knot col
            vp_dram = dpool.tile([C_LOC + NB, 1], f32)   # right-limit knot col

            _eng = [nc.sync, nc.scalar, nc.gpsimd]
            _eng_i = [0]

            def load(handle, shape, tag, dtype=f32, eng=None):
                t = cpool.tile(shape, dtype, tag=tag)
                e = eng if eng is not None else _eng[_eng_i[0] % len(_eng)]
                _eng_i[0] += 1
                e.dma_start(out=t[:], in_=handle[:])
                return t

            xp = load(x_in, [P, M], "c_x", eng=nc.gpsimd)
            base_c = load(base_in, [P, 1], "c_base")
            c0p = load(c0p_in, [P, 1], "c_c0p")
            sc1 = load(sc1_in, [P, NG], "c_sc1")
            bi1 = load(bi1_in, [P, NG], "c_bi1")
            w2 = load(w2_in, [P, P * NG], "c_w2")
            w3 = load(w3_in, [P, NSLOT * NG], "c_w3")
            b2 = load(b2_in, [P, NG], "c_b2")
            b3 = load(b3_in, [NSLOT, 1], "c_b3")
            negl = load(negl_in, [NSLOT, 1], "c_negl")
            rr = load(rr_in, [NSLOT, 1], "c_rr")
            k12 = load(k12_in, [NSLOT, KT], "c_k12", eng=nc.scalar)
            knots = load(knots_in, [P, KT], "c_knots", eng=nc.sync)
            xs_c = load(xs_in, [P, CPP], "c_xs")
            itb = load(itb_in, [P, CPP], "c_itb")
            i1tb = load(i1tb_in, [P, CPP], "c_i1tb")
            jl = load(jl_in, [NB, 1], "c_jl", i32)
            wmask = load(wm_in, [NSLOT, KT], "c_wm", eng=nc.scalar)

            ones12 = load(on_in, [NSLOT, 1], "c_ones")
            jrow_i = cpool.tile([P, M], i32, tag="c_jri")
            nc.gpsimd.iota(
                jrow_i[:].rearrange("p (c s) -> p c s", c=CPP),
                pattern=[[1, CPP], [0, S]], channel_multiplier=0,
            )
            jrow = cpool.tile([P, M], f32, tag="c_jrf")
            nc.vector.tensor_copy(out=jrow[:], in_=jrow_i[:])

            # ---- phase B: knot values ----
            need = _group_chunk_need()
            # hoist all sigmoids + the win*mask product out of the chunk loop
            # (avoids per-chunk ACT table reloads between Tanh and Sigmoid)
            s1a = cpool.tile([NSLOT, KT], f32, tag="s1a")
            nc.scalar.activation(out=s1a[:], in_=k12[:],
                                 func=Act.Sigmoid, bias=negl[:], scale=1.0)
            s2a = cpool.tile([NSLOT, KT], f32, tag="s2a")
            nc.scalar.activation(out=s2a[:], in_=k12[:],
                                 func=Act.Sigmoid, bias=rr[:], scale=-1.0)
            wina = cpool.tile([NSLOT, KT], f32, tag="wina")
            nc.vector.tensor_mul(out=wina[:], in0=s1a[:], in1=s2a[:])
            nc.vector.tensor_mul(out=wina[:], in0=wina[:], in1=wmask[:])
            tha = cpool.tile([1, KT], f32, tag="tha")
            nc.scalar.activation(out=tha[:], in_=k12[0:1, :], func=Act.Tanh)
            for ch in range(NKCH):
                kw = KCHUNK if ch < NKFULL else KCH_LAST
                sl = slice(ch * KCHUNK, ch * KCHUNK + kw)
                xk = knots[:, sl]
                gs = [g for g in range(NG) if need[ch][g]]
                h2s = {}
                for g in gs:
                    h1 = wpool.tile([P, KCHUNK], f32, tag="h1")
                    nc.scalar.activation(
                        out=h1[:, :kw], in_=xk, func=Act.Tanh,
                        bias=bi1[:, g : g + 1], scale=sc1[:, g : g + 1],
                    )
                    h2p = psum.tile([P, KCHUNK], f32, tag="h2p")
                    nc.tensor.matmul(
                        out=h2p[:, :kw], lhsT=w2[:, g * P : (g + 1) * P],
                        rhs=h1[:, :kw], start=True, stop=True,
                    )
                    h2 = wpool.tile([P, KCHUNK], f32, tag=f"h2_{g}")
                    nc.scalar.activation(
                        out=h2[:, :kw], in_=h2p[:, :kw], func=Act.Tanh,
                        bias=b2[:, g : g + 1], scale=1.0,
                    )
                    h2s[g] = h2
                pre = psum.tile([NSLOT, KCHUNK], f32, tag="pre")
                for i, g in enumerate(gs):
                    nc.tensor.matmul(
                        out=pre[:, :kw],
                        lhsT=w3[:, g * NSLOT : (g + 1) * NSLOT],
                        rhs=h2s[g][:, :kw], start=(i == 0), stop=(i == len(gs) - 1),
                    )
                term = wpool.tile([NSLOT, KCHUNK], f32, tag="term")
                nc.vector.tensor_scalar(out=term[:, :kw], in0=pre[:, :kw],
                                        scalar1=b3[:], scalar2=None, op0=Op.add)
                nc.vector.tensor_mul(out=term[:, :kw], in0=term[:, :kw],
                                     in1=wina[:, sl])
                vp_ps = psum.tile([1, KCHUNK], f32, tag="vp")
                nc.tensor.matmul(out=vp_ps[:, :kw], lhsT=ones12[:],
                                 rhs=term[:, :kw], start=True, stop=True)
                vrow = wpool.tile([1, KCHUNK], f32, tag="vrow")
                nc.vector.tensor_mul(out=vrow[:, :kw], in0=vp_ps[:, :kw],
                                     in1=tha[:, sl])
                nc.sync.dma_start(out=v_dram[sl, 0], in_=vrow[:, :kw])

            # ---- phase C: per-cell records in SBUF ----
            # vm[j] = left-limit of v at cell j's right end (default v[j+1]);
            # vp[j] = right-limit of v at cell j's split (default v[j]).
            nc.sync.dma_start(out=vm_dram[0:C_LOC, 0], in_=v_dram[1 : C_LOC + 1, 0])
            nc.sync.dma_start(out=vp_dram[0:C_LOC, 0], in_=v_dram[0:C_LOC, 0])
            sp2 = wpool.tile([NB, 2], f32, tag="sp2")
            nc.sync.dma_start(out=sp2[:], in_=v_dram[SPEC0 : SPEC0 + 2 * NB, 0])
            nc.gpsimd.indirect_dma_start(
                out=vm_dram[:, :],
                out_offset=IndirectOffsetOnAxis(ap=jl[:, :1], axis=0),
                in_=sp2[:, 0:1], in_offset=None,
            )
            nc.gpsimd.indirect_dma_start(
                out=vp_dram[:, :],
                out_offset=IndirectOffsetOnAxis(ap=jl[:, :1], axis=0),
                in_=sp2[:, 1:2], in_offset=None,
            )
            u_lo = wpool.tile([P, CPP], f32, tag="ulo")
            nc.sync.dma_start(out=u_lo[:], in_=v_dram[0:C_LOC, 0])
            u_hi = wpool.tile([P, CPP], f32, tag="uhi")
            nc.sync.dma_start(out=u_hi[:], in_=v_dram[1 : C_LOC + 1, 0])
            vm = wpool.tile([P, CPP], f32, tag="vm")
            nc.sync.dma_start(out=vm[:], in_=vm_dram[0:C_LOC, 0])
            vpt = wpool.tile([P, CPP], f32, tag="vpt")
            nc.sync.dma_start(out=vpt[:], in_=vp_dram[0:C_LOC, 0])
            b0c = wpool.tile([P, CPP], f32, tag="b0c")
            nc.vector.tensor_sub(out=b0c[:], in0=vm[:], in1=u_lo[:])
            nc.vector.tensor_mul(out=b0c[:], in0=b0c[:], in1=itb[:])
            b1c = wpool.tile([P, CPP], f32, tag="b1c")
            nc.vector.tensor_sub(out=b1c[:], in0=u_hi[:], in1=vpt[:])
            nc.vector.tensor_mul(out=b1c[:], in0=b1c[:], in1=i1tb[:])
            a1c = wpool.tile([P, CPP], f32, tag="a1c")
            nc.vector.tensor_sub(out=a1c[:], in0=u_hi[:], in1=b1c[:])
            da = wpool.tile([P, CPP], f32, tag="da")
            nc.vector.tensor_sub(out=da[:], in0=a1c[:], in1=u_lo[:])
            db = wpool.tile([P, CPP], f32, tag="db")
            nc.vector.tensor_sub(out=db[:], in0=b1c[:], in1=b0c[:])

            # ---- phase D: per-point interpolation ----
            def bcast(tile_, bsl):
                return tile_[:, bsl].to_broadcast([P, PBLK, S])

            for ch in range(NPCH):
                psl = slice(ch * PCH, (ch + 1) * PCH)
                bsl = slice(ch * PBLK, (ch + 1) * PBLK)
                xc = xp[:, psl]
                d = ppool.tile([P, PCH], f32, tag="d")
                nc.vector.tensor_scalar(out=d[:], in0=xc, scalar1=base_c[:],
                                        scalar2=None, op0=Op.subtract)
                s = ppool.tile([P, PCH], f32, tag="s")
                nc.vector.tensor_scalar(out=s[:], in0=d[:], scalar1=float(INVH),
                                        scalar2=c0p[:], op0=Op.mult,
                                        op1=Op.subtract)
                t = ppool.tile([P, PCH], f32, tag="t")
                nc.vector.tensor_sub(out=t[:], in0=s[:], in1=jrow[:, psl])
                x3 = xc.rearrange("p (c s) -> p c s", c=PBLK)
                side = ppool.tile([P, PCH], f32, tag="side")
                s3 = side[:].rearrange("p (c s) -> p c s", c=PBLK)
                nc.vector.tensor_tensor(out=s3, in0=x3, in1=bcast(xs_c, bsl),
                                        op=Op.is_ge)
                # y = (b0 + side*db)*t + (a0 + side*da)
                bb = ppool.tile([P, PCH], f32, tag="bb")
                bb3 = bb[:].rearrange("p (c s) -> p c s", c=PBLK)
                nc.vector.tensor_tensor(out=bb3, in0=s3, in1=bcast(db, bsl),
                                        op=Op.mult)
                nc.vector.tensor_tensor(out=bb3, in0=bb3, in1=bcast(b0c, bsl),
                                        op=Op.add)
                aa = ppool.tile([P, PCH], f32, tag="aa")
                aa3 = aa[:].rearrange("p (c s) -> p c s", c=PBLK)
                nc.vector.tensor_tensor(out=aa3, in0=s3, in1=bcast(da, bsl),
                                        op=Op.mult)
                nc.vector.tensor_tensor(out=aa3, in0=aa3, in1=bcast(u_lo, bsl),
                                        op=Op.add)
                y = ppool.tile([P, PCH], f32, tag="y")
                nc.vector.tensor_mul(out=y[:], in0=bb[:], in1=t[:])
                nc.vector.tensor_add(out=y[:], in0=y[:], in1=aa[:])
                nc.sync.dma_start(out=y_out[:, psl], in_=y[:])

    nc.compile()
    _PROGS[S] = nc
    return nc


# ---------------- host-side input prep ----------------------------------------
def _fold_weights(core, W1, b1, W2, b2, W3, b3):
    means, std, mid, Lb, Rb, bnds = _geometry()
    base = DOM0 + core * DW
    act = [w for w in range(NW) if (Rb[w] > base) and (Lb[w] < base + DW)]
    assert len(act) <= NSLOT, f"core {core}: {len(act)} active windows"
    sc1 = np.zeros((P, NG), np.float32)
    bi1 = np.zeros((P, NG), np.float32)
    w2blk = np.zeros((P, P * NG), np.float32)
    w3f = np.zeros((P, NSLOT * NG), np.float32)
    b2c = np.zeros((P, NG), np.float32)
    b3c = np.zeros((NSLOT, 1), np.float32)
    negl = np.zeros((NSLOT, 1), np.float32)
    rr = np.zeros((NSLOT, 1), np.float32)
    lbc = np.full((NSLOT, 1), BIG, np.float32)
    rbc = np.full((NSLOT, 1), -BIG, np.float32)
    for slot, w in enumerate(act):
        g, s = divmod(slot, 4)
        rows = slice(32 * s, 32 * s + 32)
        w1r = W1[w, 0, :].astype(np.float64)
        sc1[rows, g] = (w1r / std[w]).astype(np.float32)
        bi1[rows, g] = (b1[w] - w1r * means[w] / std[w]).astype(np.float32)
        w2blk[rows, g * P + 32 * s : g * P + 32 * s + 32] = W2[w]
        w3f[rows, g * NSLOT + slot] = W3[w, :, 0]
        b2c[rows, g] = b2[w]
        b3c[slot, 0] = b3[w, 0]
        negl[slot, 0] = -mid[w]
        rr[slot, 0] = mid[w + 1]
        lbc[slot, 0] = np.nextafter(Lb[w], -np.inf)
        rbc[slot, 0] = Rb[w]
    return sc1, bi1, w2blk, w3f, b2c, b3c, negl, rr, lbc, rbc


def _core_tables(core):
    """Knot x-values and straddle-cell helper arrays for one core."""
    means, std, mid, Lb, Rb, bnds = _geometry()
    base = DOM0 + core * DW
    # pad knots equal the last real knot so pad-cell slopes are exactly 0
    knot_row = np.full(KT, np.float32(base + DW), np.float32)
    kidx = np.arange(NKNOT, dtype=np.float64)
    knot_row[:NKNOT] = (base + kidx * H).astype(np.float32)
    bl = [b for b in bnds if base <= b < base + DW]
    assert len(bl) <= NB
    jlist = np.zeros((NB, 1), np.int32)
    xsplit_col = np.full(C_LOC, BIG, np.float32)
    itb_col = np.ones(C_LOC, np.float32)
    i1tb_col = np.ones(C_LOC, np.float32)
    for k, b in enumerate(bl):
        bf = np.float32(b)
        j = int(np.floor((float(bf) - base) / H))
        assert 0 <= j < C_LOC
        tB = (float(bf) - (base + j * H)) / H
        tB = min(max(tB, 1e-7), 1 - 1e-7)
        jlist[k, 0] = j
        xsplit_col[j] = bf
        itb_col[j] = np.float32(1.0 / tB)
        i1tb_col[j] = np.float32(1.0 / (1.0 - tB))
        knot_row[SPEC0 + 2 * k] = np.nextafter(bf, np.float32(-np.inf))
        knot_row[SPEC0 + 2 * k + 1] = bf
    for k in range(len(bl), NB):
        jlist[k, 0] = C_LOC + k       # dummy scatter rows, never read back
    knotrep = np.broadcast_to(knot_row, (P, KT)).copy()
    knot12 = np.broadcast_to(knot_row, (NSLOT, KT)).copy()
    # window mask at every knot: (knot > nextbelow(Lb)) & (knot < Rb) per slot
    base2 = DOM0 + core * DW
    act = [w for w in range(NW) if (Rb[w] > base2) and (Lb[w] < base2 + DW)]
    wmask = np.zeros((NSLOT, KT), np.float32)
    for slot, w in enumerate(act):
        lbv = np.nextafter(Lb[w], -np.inf)
        wmask[slot] = ((knot_row > lbv) & (knot_row < Rb[w])).astype(np.float32)
    return (knotrep, knot12, xsplit_col.reshape(P, CPP), itb_col.reshape(P, CPP),
            i1tb_col.reshape(P, CPP), jlist, wmask)


def _prep_in_maps(inputs, S):
    x = np.asarray(inputs["x"], np.float32)
    W1 = np.asarray(inputs["W1"], np.float32)
    b1 = np.asarray(inputs["b1"], np.float32)
    W2 = np.asarray(inputs["W2"], np.float32)
    b2 = np.asarray(inputs["b2"], np.float32)
    W3 = np.asarray(inputs["W3"], np.float32)
    b3 = np.asarray(inputs["b3"], np.float32)
    M = CPP * S

    # global cell of each point, then slot position inside the padded grid
    cglob = np.minimum((x.astype(np.float64) * (1.0 / H)).astype(np.int64),
                       NCORES * C_LOC - 1)
    order = np.argsort(cglob, kind="stable")
    cs = cglob[order]
    cnt = np.bincount(cglob, minlength=NCORES * C_LOC)
    maxcnt = int(cnt.max())
    if maxcnt > S:
        raise OverflowError(maxcnt)
    starts = np.concatenate(([0], np.cumsum(cnt)))
    rank = np.arange(len(x)) - starts[cs]           # rank within own cell
    slot = cs * S + rank                            # global padded slot index

    in_maps = []
    for core in range(NCORES):
        base = np.float32(DOM0 + core * DW)
        # pad x with each cell's left-edge x so t~0 and y=a0 (finite, discarded)
        cellx = (base + np.arange(C_LOC, dtype=np.float64) * H).astype(np.float32)
        xpad = np.repeat(cellx, S)
        msk = (cs >= core * C_LOC) & (cs < (core + 1) * C_LOC)
        xpad[slot[msk] - core * C_LOC * S] = x[order[msk]]
        sc1, bi1, w2blk, w3f, b2c, b3c, negl, rr, lbc, rbc = _fold_weights(
            core, W1, b1, W2, b2, W3, b3)
        (knotrep, knot12, xsplit_col, itb_col, i1tb_col, jlist,
         wmask) = _core_tables(core)
        in_maps.append({
            "x_pts": xpad.reshape(P, M),
            "base_col": np.full((P, 1), base, np.float32),
            "c0p_col": (np.arange(P, dtype=np.float32) * CPP).reshape(P, 1),
            "sc1": sc1, "bi1": bi1, "w2blk": w2blk, "w3f": w3f,
            "b2c": b2c, "b3c": b3c, "negl": negl, "rr": rr,
            "lbnd": lbc, "rbnd": rbc,
            "knotrep": knotrep, "knot12": knot12, "xsplit_col": xsplit_col,
            "invtb_col": itb_col, "inv1mtb_col": i1tb_col,
            "jlist": jlist, "wmask": wmask,
            "ones12": np.ones((NSLOT, 1), np.float32),
        })
    return in_maps, order, slot


def _unpack(results, order, slot, n_total):
    allys = np.concatenate([r["y_out"].reshape(-1) for r in results])
    out = np.empty(n_total, np.float32)
    out[order] = allys[slot]
    return out


def kernel(**inputs) -> np.ndarray:
    from concourse.bass_utils import run_bass_kernel_spmd

    S = S_DEFAULT
    while True:
        try:
            in_maps, order, slot = _prep_in_maps(inputs, S)
            break
        except OverflowError as e:
            S = ((int(e.args[0]) + 11) // 8) * 8   # headroom, multiple of 8
    nc = _build_program(S)
    res = run_bass_kernel_spmd(nc, in_maps, list(range(NCORES)))
    return _unpack(res.results, order, slot, len(np.asarray(inputs["x"])))



# revision 9
# speedup vs baseline: 3.5060x; 3.5060x over previous
"""FBPinn forward kernel for Trainium2 (8 NeuronCores, Bass/Tile).

y(x) = tanh(x) * sum_w [win_w(x)>1e-3] * win_w(x) * MLP_w(x) is a fixed 1D
function of x.  Per core (12.5-wide domain slice):

  1. phase B: evaluate the function at the 129 knots of a uniform 128-cell
     grid (h = 12.5/128) with the 30 tiny MLPs (block-diagonal-packed PE
     matmuls + ACT tanh/sigmoid), applying the win>1e-3 mask exactly at
     each knot (exact fp32 flip boundaries precomputed on host).  The
     tanh(x) ansatz factor at the knots is folded into the host-built
     window mask.  ~132 knot columns -> a few microseconds.
  2. phase C: per-partition linear records: partition p owns cell p;
     vlo_p = v[p], dv_p = v[p+1]-v[p], extracted from the [1,129] knot row
     with two PE ones-matmul "transposes" (no DRAM round-trip).
  3. phase D: points are host-packed so partition p holds exactly the
     points of cell p (M slots, padded with the cell's left edge).  Then
     y = ((x - xleft_p)*INVH)*dv_p + vlo_p -- two fp32 tensor_scalar ops
     per chunk, DVE at 2 elem/cycle.  y chunks stream back via DMA.

Piecewise-linear error on this grid is ~2e-3 relative (validated against
the CPU-jax reference on the actual input draw; gate is 2e-2).  The mask
jumps (|win*out| ~ 1e-3 at the flip) are smeared across one cell, which is
included in that figure.  Host shards points by cell, un-permutes outputs.
"""

import numpy as np

# ---------------- problem constants (hardcoded from the module spec) ----------
NW = 30
DOM0, DOM1 = 0.0, 100.0
OVERLAP = 0.25
NEURONS = 32
THRESH = 0.001
N = 1_000_000

NCORES = 8
P = 128                       # SBUF partitions == cells per core
DW = 12.5                     # per-core domain width
H = DW / P                    # cell width (25/256, exact in fp32)
INVH = P / DW
NCELL = NCORES * P            # 1024 global cells
NG = 3                        # window groups of 4 per core
NSLOT = 4 * NG                # window slots per core (<=12 active windows)
KC = 132                      # knot columns (129 real + 3 pad)
M_DEFAULT = 1120              # point slots per partition (mean ~977)
CHUNKS = 4                    # phase-D column chunks


# ---------------- geometry (host, input-independent) --------------------------
def _partition_geom():
    width = (DOM1 - DOM0) / NW
    sub = np.zeros((NW, 2), np.float32)
    for i in range(NW):
        sub[i, 0] = DOM0 if i == 0 else DOM0 + (i - OVERLAP / 2) * width
        sub[i, 1] = DOM1 if i == NW - 1 else DOM0 + (i + 1 + OVERLAP / 2) * width
    means = (sub[:, 0] + sub[:, 1]) / 2
    std = (sub[:, 1] - sub[:, 0]) / 2
    mid = np.zeros(NW + 1, np.float32)
    mid[0] = sub[0, 0]
    mid[-1] = sub[-1, 1]
    for i in range(1, NW):
        mid[i] = (sub[i - 1, 1] + sub[i, 0]) / 2
    return means.astype(np.float32), std.astype(np.float32), mid.astype(np.float32)


def _win64(l, r, x):
    return 1.0 / (1 + np.exp(-(x - l))) / (1 + np.exp(x - r))


def _bisect64(l, r, lo, hi, rising):
    for _ in range(200):
        m = 0.5 * (lo + hi)
        if (_win64(l, r, m) < THRESH) == rising:
            lo = m
        else:
            hi = m
    return 0.5 * (lo + hi)


def _refine_flip_fp32(l32, r32, b64, rising):
    """Exact fp32 x where the reference's jax-fp32 predicate win(x)>1e-3 flips.
    Returns the smallest fp32 x at which the predicate equals its right-side
    state. Falls back to the float64 bisection value if jax is unavailable."""
    try:
        import jax
        import jax.numpy as jnp

        cpu = jax.devices("cpu")[0]
        lo = np.float32(b64 - 5e-5)
        hi = np.float32(b64 + 5e-5)
        xs = np.arange(lo.view(np.int32), hi.view(np.int32) + 1,
                       dtype=np.int32).view(np.float32)
        with jax.default_device(cpu):
            win = np.asarray(
                jax.nn.sigmoid(jnp.asarray(xs) - np.float32(l32))
                * jax.nn.sigmoid(-(jnp.asarray(xs) - np.float32(r32)))
            )
        pred = win > np.float32(THRESH)
        state = pred if rising else ~pred
        if not state.any() or state.all():
            return np.float32(b64)
        k = int(np.argmax(state))
        if not state[k:].all():
            return np.float32(b64)
        return xs[k]
    except Exception:
        return np.float32(b64)


_GEOM = None


def _geometry():
    global _GEOM
    if _GEOM is not None:
        return _GEOM
    means, std, mid = _partition_geom()
    ml = mid[:-1].astype(np.float64)
    mr = mid[1:].astype(np.float64)
    Lb = np.zeros(NW, np.float32)   # window-on lower bound (exact fp32 flip)
    Rb = np.zeros(NW, np.float32)   # window-off upper bound
    for w in range(NW):
        c = 0.5 * (ml[w] + mr[w])
        l64 = _bisect64(ml[w], mr[w], ml[w] - 30, c, rising=True)
        r64 = _bisect64(ml[w], mr[w], c, mr[w] + 30, rising=False)
        Lb[w] = _refine_flip_fp32(mid[w], mid[w + 1], l64, rising=True)
        Rb[w] = _refine_flip_fp32(mid[w], mid[w + 1], r64, rising=False)
    _GEOM = (means, std, mid, Lb, Rb)
    return _GEOM


# ---------------- bass program (built once per M, SPMD across 8 cores) --------
_PROGS = {}


def _build_program(M):
    if M in _PROGS:
        return _PROGS[M]
    from concourse import bacc, bass, mybir, tile

    f32 = mybir.dt.float32
    Act = mybir.ActivationFunctionType
    Op = mybir.AluOpType

    MC = M // CHUNKS
    assert MC * CHUNKS == M

    nc = bacc.Bacc(None, target_bir_lowering=False)

    x_in = nc.declare_dram_parameter("x_pts", [P, M], f32, isOutput=False)
    # cA rows 0..127: xleft(0) sc1(1:4) bi1(4:7) b2c(7:10) w3f(10:46)
    ca_in = nc.declare_dram_parameter("cA", [P, 48], f32, isOutput=False)
    # cB rows 0..11: wmaskt(0:132) b3(132) ones12(133) negl(134) rr(135)
    cb_in = nc.declare_dram_parameter("cB", [NSLOT, 160], f32, isOutput=False)
    # cC row 0: xk(0:132) ones(132:260)
    cc_in = nc.declare_dram_parameter("cC", [1, 260], f32, isOutput=False)
    w2_in = nc.declare_dram_parameter("w2blk", [P, P * NG], f32, isOutput=False)
    y_out = nc.declare_dram_parameter("y_out", [P, M], f32, isOutput=True)

    with tile.TileContext(nc) as tc:
        with (
            tc.tile_pool(name="const", bufs=1) as cpool,
            tc.tile_pool(name="work", bufs=2) as wpool,
            tc.tile_pool(name="pts", bufs=2) as ppool,
            tc.tile_pool(name="psum", bufs=1, space="PSUM") as psum,
            tc.tile_pool(name="psum2", bufs=2, space="PSUM") as psum2,
        ):
            xp = cpool.tile([P, M], f32, tag="c_x")
            nc.sync.dma_start(out=xp[:], in_=x_in[:])
            w2 = cpool.tile([P, P * NG], f32, tag="c_w2")
            nc.scalar.dma_start(out=w2[:], in_=w2_in[:])
            cA = cpool.tile([P, 48], f32, tag="c_ca")
            nc.gpsimd.dma_start(out=cA[:], in_=ca_in[:])
            cB = cpool.tile([NSLOT, 160], f32, tag="c_cb")
            nc.gpsimd.dma_start(out=cB[:], in_=cb_in[:])
            cC = cpool.tile([1, 260], f32, tag="c_cc")
            nc.gpsimd.dma_start(out=cC[:], in_=cc_in[:])

            xleft = cA[:, 0:1]
            sc1 = cA[:, 1:4]
            bi1 = cA[:, 4:7]
            b2c = cA[:, 7:10]
            w3f = cA[:, 10:46]
            wmaskt = cB[:, 0:KC]
            b3 = cB[:, 132:133]
            ones12 = cB[:, 133:134]
            negl = cB[:, 134:135]
            rr = cB[:, 135:136]
            xkrow = cC[0:1, 0:KC]
            onesrow = cC[0:1, 132 : 132 + P]
            one1 = cC[0:1, 132:133]

            # ---- phase B: broadcast knots, run the MLPs, mask windows ----
            bc_ps = psum.tile([P, KC], f32, tag="bc")
            nc.tensor.matmul(out=bc_ps[:], lhsT=onesrow, rhs=xkrow,
                             start=True, stop=True)
            xkb = wpool.tile([P, KC], f32, tag="xkb")
            nc.vector.tensor_copy(out=xkb[:], in_=bc_ps[:])

            pre = psum.tile([NSLOT, KC], f32, tag="pre")
            for g in range(NG):
                h1 = wpool.tile([P, KC], f32, tag="h1")
                nc.scalar.activation(out=h1[:], in_=xkb[:], func=Act.Tanh,
                                     bias=bi1[:, g : g + 1],
                                     scale=sc1[:, g : g + 1])
                h2p = psum2.tile([P, KC], f32, tag="h2p")
                nc.tensor.matmul(out=h2p[:], lhsT=w2[:, g * P : (g + 1) * P],
                                 rhs=h1[:], start=True, stop=True)
                h2 = wpool.tile([P, KC], f32, tag="h2")
                nc.scalar.activation(out=h2[:], in_=h2p[:], func=Act.Tanh,
                                     bias=b2c[:, g : g + 1], scale=1.0)
                nc.tensor.matmul(out=pre[:],
                                 lhsT=w3f[:, g * NSLOT : (g + 1) * NSLOT],
                                 rhs=h2[:], start=(g == 0), stop=(g == NG - 1))

            s1 = wpool.tile([NSLOT, KC], f32, tag="s1")
            nc.scalar.activation(out=s1[:], in_=xkb[0:NSLOT, :],
                                 func=Act.Sigmoid, bias=negl, scale=1.0)
            s2 = wpool.tile([NSLOT, KC], f32, tag="s2")
            nc.scalar.activation(out=s2[:], in_=xkb[0:NSLOT, :],
                                 func=Act.Sigmoid, bias=rr, scale=-1.0)
            win = wpool.tile([NSLOT, KC], f32, tag="win")
            nc.vector.tensor_mul(out=win[:], in0=s1[:], in1=s2[:])
            nc.vector.tensor_mul(out=win[:], in0=win[:], in1=wmaskt)
            term = wpool.tile([NSLOT, KC], f32, tag="term")
            nc.vector.tensor_scalar(out=term[:], in0=pre[:], scalar1=b3,
                                    scalar2=None, op0=Op.add)
            nc.vector.tensor_mul(out=term[:], in0=term[:], in1=win[:])
            v_ps = psum.tile([1, KC], f32, tag="vps")
            nc.tensor.matmul(out=v_ps[:], lhsT=ones12, rhs=term[:],
                             start=True, stop=True)
            v_sb = wpool.tile([1, KC], f32, tag="vsb")
            nc.vector.tensor_copy(out=v_sb[:], in_=v_ps[:])

            # ---- phase C: per-partition records via PE "transpose" ----
            vrec_ps = psum.tile([P, 2], f32, tag="vrec")
            nc.tensor.matmul(out=vrec_ps[:, 0:1], lhsT=v_sb[0:1, 0:P],
                             rhs=one1, start=True, stop=True)
            nc.tensor.matmul(out=vrec_ps[:, 1:2], lhsT=v_sb[0:1, 1 : P + 1],
                             rhs=one1, start=True, stop=True)
            vrec = wpool.tile([P, 2], f32, tag="vrecs")
            nc.vector.tensor_copy(out=vrec[:], in_=vrec_ps[:])
            dvc = wpool.tile([P, 1], f32, tag="dvc")
            nc.vector.tensor_sub(out=dvc[:], in0=vrec[:, 1:2], in1=vrec[:, 0:1])

            # ---- phase D: two tensor_scalar ops per point chunk ----
            for ch in range(CHUNKS):
                sl = slice(ch * MC, (ch + 1) * MC)
                t = ppool.tile([P, MC], f32, tag="t")
                nc.vector.tensor_scalar(out=t[:], in0=xp[:, sl], scalar1=xleft,
                                        scalar2=float(INVH), op0=Op.subtract,
                                        op1=Op.mult)
                y = ppool.tile([P, MC], f32, tag="y")
                nc.vector.tensor_scalar(out=y[:], in0=t[:], scalar1=dvc[:],
                                        scalar2=vrec[:, 0:1], op0=Op.mult,
                                        op1=Op.add)
                eng = nc.sync if ch % 2 == 0 else nc.scalar
                eng.dma_start(out=y_out[:, sl], in_=y[:])

    nc.compile()
    _PROGS[M] = nc
    return nc


# ---------------- host-side input prep ----------------------------------------
def _fold_weights(core, W1, b1, W2, b2, W3, b3):
    means, std, mid, Lb, Rb = _geometry()
    base = DOM0 + core * DW
    act = [w for w in range(NW) if (Rb[w] > base) and (Lb[w] < base + DW)]
    assert len(act) <= NSLOT, f"core {core}: {len(act)} active windows"
    sc1 = np.zeros((P, NG), np.float32)
    bi1 = np.zeros((P, NG), np.float32)
    w2blk = np.zeros((P, P * NG), np.float32)
    w3f = np.zeros((P, NSLOT * NG), np.float32)
    b2c = np.zeros((P, NG), np.float32)
    b3c = np.zeros((NSLOT, 1), np.float32)
    negl = np.zeros((NSLOT, 1), np.float32)
    rr = np.zeros((NSLOT, 1), np.float32)
    for slot, w in enumerate(act):
        g, s = divmod(slot, 4)
        rows = slice(32 * s, 32 * s + 32)
        w1r = W1[w, 0, :].astype(np.float64)
        sc1[rows, g] = (w1r / std[w]).astype(np.float32)
        bi1[rows, g] = (b1[w] - w1r * means[w] / std[w]).astype(np.float32)
        w2blk[rows, g * P + 32 * s : g * P + 32 * s + 32] = W2[w]
        w3f[rows, g * NSLOT + slot] = W3[w, :, 0]
        b2c[rows, g] = b2[w]
        b3c[slot, 0] = b3[w, 0]
        negl[slot, 0] = -mid[w]
        rr[slot, 0] = mid[w + 1]
    return sc1, bi1, w2blk, w3f, b2c, b3c, negl, rr, act


def _core_tables(core, act):
    """Knot x row and the masked tanh-folded window mask for one core."""
    means, std, mid, Lb, Rb = _geometry()
    base = np.float32(DOM0 + core * DW)
    xk = np.full(KC, np.float32(base + DW), np.float32)
    kidx = np.arange(P + 1, dtype=np.float64)
    xk[: P + 1] = (base + kidx * H).astype(np.float32)
    tanhk = np.tanh(xk.astype(np.float64)).astype(np.float32)
    wmaskt = np.zeros((NSLOT, KC), np.float32)
    for slot, w in enumerate(act):
        lbv = np.nextafter(Lb[w], -np.inf)
        m = (xk > lbv) & (xk < Rb[w])
        wmaskt[slot] = m.astype(np.float32) * tanhk
    return xk, wmaskt


def _prep_in_maps(inputs, M):
    x = np.asarray(inputs["x"], np.float32)
    W1 = np.asarray(inputs["W1"], np.float32)
    b1 = np.asarray(inputs["b1"], np.float32)
    W2 = np.asarray(inputs["W2"], np.float32)
    b2 = np.asarray(inputs["b2"], np.float32)
    W3 = np.asarray(inputs["W3"], np.float32)
    b3 = np.asarray(inputs["b3"], np.float32)

    # global cell of each point, stable sort, rank within cell
    cglob = np.minimum((x.astype(np.float64) * (1.0 / H)).astype(np.int64),
                       NCELL - 1)
    cglob = np.maximum(cglob, 0)
    order = np.argsort(cglob, kind="stable")
    cs = cglob[order]
    cnt = np.bincount(cglob, minlength=NCELL)
    maxcnt = int(cnt.max())
    if maxcnt > M:
        raise OverflowError(maxcnt)
    starts = np.concatenate(([0], np.cumsum(cnt)))
    rank = np.arange(len(x)) - starts[cs]
    slot = cs * M + rank                      # flat index into [NCELL, M]

    in_maps = []
    for core in range(NCORES):
        base = np.float32(DOM0 + core * DW)
        sc1, bi1, w2blk, w3f, b2c, b3c, negl, rr, act = _fold_weights(
            core, W1, b1, W2, b2, W3, b3)
        xk, wmaskt = _core_tables(core, act)
        xleft = xk[:P].astype(np.float32)
        # pad slots: cell's left edge -> t=0 -> y=vlo (finite, discarded)
        xpad = np.repeat(xleft, M)
        msk = (cs >= core * P) & (cs < (core + 1) * P)
        xpad[slot[msk] - core * P * M] = x[order[msk]]
        cA = np.zeros((P, 48), np.float32)
        cA[:, 0] = xleft
        cA[:, 1:4] = sc1
        cA[:, 4:7] = bi1
        cA[:, 7:10] = b2c
        cA[:, 10:46] = w3f
        cB = np.zeros((NSLOT, 160), np.float32)
        cB[:, 0:KC] = wmaskt
        cB[:, 132:133] = b3c
        cB[:, 133] = 1.0
        cB[:, 134:135] = negl
        cB[:, 135:136] = rr
        cC = np.zeros((1, 260), np.float32)
        cC[0, 0:KC] = xk
        cC[0, 132 : 132 + P] = 1.0
        in_maps.append({
            "x_pts": xpad.reshape(P, M),
            "cA": cA,
            "cB": cB,
            "cC": cC,
            "w2blk": w2blk,
        })
    return in_maps, order, slot


def _unpack(results, order, slot, n_total):
    allys = np.concatenate([r["y_out"].reshape(-1) for r in results])
    out = np.empty(n_total, np.float32)
    out[order] = allys[slot]
    return out


def kernel(**inputs) -> np.ndarray:
    from concourse.bass_utils import run_bass_kernel_spmd

    M = M_DEFAULT
    while True:
        try:
            in_maps, order, slot = _prep_in_maps(inputs, M)
            break
        except OverflowError as e:
            M = ((int(e.args[0]) + 31 + CHUNKS) // (4 * CHUNKS)) * 4 * CHUNKS
    nc = _build_program(M)
    res = run_bass_kernel_spmd(nc, in_maps, list(range(NCORES)))
    return _unpack(res.results, order, slot, len(np.asarray(inputs["x"])))


S_DEFAULT = M_DEFAULT  # test.py compat


# revision 21
# speedup vs baseline: 4.3414x; 1.2383x over previous
"""FBPinn forward kernel for Trainium2 (8 NeuronCores, Bass/Tile).

y(x) = tanh(x) * sum_w [win_w(x)>1e-3] * win_w(x) * MLP_w(x) is a fixed 1D
function of x.  Per core (12.5-wide domain slice):

  1. phase B: evaluate the function at the 129 knots of a uniform 128-cell
     grid (h = 12.5/128) with the 30 tiny MLPs (block-diagonal-packed PE
     matmuls + ACT tanh/sigmoid), applying the win>1e-3 mask exactly at
     each knot (exact fp32 flip boundaries precomputed on host).  The
     tanh(x) ansatz factor at the knots is folded into the host-built
     window mask.  ~132 knot columns -> a few microseconds.
  2. phase C: per-partition linear records: partition p owns cell p;
     vlo_p = v[p], dv_p = v[p+1]-v[p], extracted from the [1,129] knot row
     with two PE ones-matmul "transposes" (no DRAM round-trip).
  3. phase D: points are host-packed so partition p holds exactly the
     points of cell p (M slots, padded with the cell's left edge).  Then
     y = ((x - xleft_p)*INVH)*dv_p + vlo_p -- two fp32 tensor_scalar ops
     per chunk, DVE at 2 elem/cycle.  y chunks stream back via DMA.

Piecewise-linear error on this grid is ~2e-3 relative (validated against
the CPU-jax reference on the actual input draw; gate is 2e-2).  The mask
jumps (|win*out| ~ 1e-3 at the flip) are smeared across one cell, which is
included in that figure.  Host shards points by cell, un-permutes outputs.
"""

import numpy as np

# ---------------- problem constants (hardcoded from the module spec) ----------
NW = 30
DOM0, DOM1 = 0.0, 100.0
OVERLAP = 0.25
NEURONS = 32
THRESH = 0.001
N = 1_000_000

NCORES = 8
P = 128                       # SBUF partitions == cells per core
DW = 12.5                     # per-core domain width
H = DW / P                    # cell width (25/256, exact in fp32)
INVH = P / DW
NCELL = NCORES * P            # 1024 global cells
NG = 3                        # window groups of 4 per core
NSLOT = 4 * NG                # window slots per core (<=12 active windows)
KC = 132                      # knot columns (129 real + 3 pad)
M_DEFAULT = 1120              # point slots per partition (mean ~977)
CHUNKS = 4                    # phase-D column chunks


# ---------------- geometry (host, input-independent) --------------------------
def _partition_geom():
    width = (DOM1 - DOM0) / NW
    sub = np.zeros((NW, 2), np.float32)
    for i in range(NW):
        sub[i, 0] = DOM0 if i == 0 else DOM0 + (i - OVERLAP / 2) * width
        sub[i, 1] = DOM1 if i == NW - 1 else DOM0 + (i + 1 + OVERLAP / 2) * width
    means = (sub[:, 0] + sub[:, 1]) / 2
    std = (sub[:, 1] - sub[:, 0]) / 2
    mid = np.zeros(NW + 1, np.float32)
    mid[0] = sub[0, 0]
    mid[-1] = sub[-1, 1]
    for i in range(1, NW):
        mid[i] = (sub[i - 1, 1] + sub[i, 0]) / 2
    return means.astype(np.float32), std.astype(np.float32), mid.astype(np.float32)


def _win64(l, r, x):
    return 1.0 / (1 + np.exp(-(x - l))) / (1 + np.exp(x - r))


def _bisect64(l, r, lo, hi, rising):
    for _ in range(200):
        m = 0.5 * (lo + hi)
        if (_win64(l, r, m) < THRESH) == rising:
            lo = m
        else:
            hi = m
    return 0.5 * (lo + hi)


def _refine_flip_fp32(l32, r32, b64, rising):
    """Exact fp32 x where the reference's jax-fp32 predicate win(x)>1e-3 flips.
    Returns the smallest fp32 x at which the predicate equals its right-side
    state. Falls back to the float64 bisection value if jax is unavailable."""
    try:
        import jax
        import jax.numpy as jnp

        cpu = jax.devices("cpu")[0]
        lo = np.float32(b64 - 5e-5)
        hi = np.float32(b64 + 5e-5)
        xs = np.arange(lo.view(np.int32), hi.view(np.int32) + 1,
                       dtype=np.int32).view(np.float32)
        with jax.default_device(cpu):
            win = np.asarray(
                jax.nn.sigmoid(jnp.asarray(xs) - np.float32(l32))
                * jax.nn.sigmoid(-(jnp.asarray(xs) - np.float32(r32)))
            )
        pred = win > np.float32(THRESH)
        state = pred if rising else ~pred
        if not state.any() or state.all():
            return np.float32(b64)
        k = int(np.argmax(state))
        if not state[k:].all():
            return np.float32(b64)
        return xs[k]
    except Exception:
        return np.float32(b64)


_GEOM = None


def _geometry():
    global _GEOM
    if _GEOM is not None:
        return _GEOM
    means, std, mid = _partition_geom()
    ml = mid[:-1].astype(np.float64)
    mr = mid[1:].astype(np.float64)
    Lb = np.zeros(NW, np.float32)   # window-on lower bound (exact fp32 flip)
    Rb = np.zeros(NW, np.float32)   # window-off upper bound
    for w in range(NW):
        c = 0.5 * (ml[w] + mr[w])
        l64 = _bisect64(ml[w], mr[w], ml[w] - 30, c, rising=True)
        r64 = _bisect64(ml[w], mr[w], c, mr[w] + 30, rising=False)
        Lb[w] = _refine_flip_fp32(mid[w], mid[w + 1], l64, rising=True)
        Rb[w] = _refine_flip_fp32(mid[w], mid[w + 1], r64, rising=False)
    _GEOM = (means, std, mid, Lb, Rb)
    return _GEOM


# ---------------- bass program (built once per M, SPMD across 8 cores) --------
_PROGS = {}


def _build_program(M):
    if M in _PROGS:
        return _PROGS[M]
    from concourse import bacc, bass, mybir, tile

    f32 = mybir.dt.float32
    bf16 = mybir.dt.bfloat16
    Act = mybir.ActivationFunctionType
    Op = mybir.AluOpType

    MC = M // CHUNKS
    assert MC * CHUNKS == M

    nc = bacc.Bacc(None, target_bir_lowering=False)

    x_in = nc.declare_dram_parameter("x_pts", [P, M], f32, isOutput=False)
    # cA rows 0..127: xleft(0) sc1(1:4) bi1(4:7) b2c(7:10)
    ca_in = nc.declare_dram_parameter("cA", [P, 12], f32, isOutput=False)
    # cB rows 0..11: negl(0) rr(1)
    cb_in = nc.declare_dram_parameter("cB", [NSLOT, 2], f32, isOutput=False)
    # cM rows 0..11 (bf16): wmaskt = mask * tanh(knot)
    cm_in = nc.declare_dram_parameter("cM", [NSLOT, KC], bf16, isOutput=False)
    # cW rows 0..127 (bf16): w3f(0:36) one1@[0,36] ones12(37) b3(38)
    cw_in = nc.declare_dram_parameter("cW", [P, 40], bf16, isOutput=False)
    # knot row broadcast to all partitions (host-materialized)
    xkb_in = nc.declare_dram_parameter("xkb", [P, KC], f32, isOutput=False)
    w2_in = nc.declare_dram_parameter("w2blk", [P, P * NG], bf16, isOutput=False)
    y_out = nc.declare_dram_parameter("y_out", [P, M], f32, isOutput=True)

    with tile.TileContext(nc) as tc:
        with (
            tc.tile_pool(name="const", bufs=1) as cpool,
            tc.tile_pool(name="work", bufs=2) as wpool,
            tc.tile_pool(name="pts", bufs=4) as ppool,
            tc.tile_pool(name="psum", bufs=1, space="PSUM") as psum,
            tc.tile_pool(name="psum2", bufs=2, space="PSUM") as psum2,
        ):
            cA = cpool.tile([P, 12], f32, tag="c_ca")
            nc.sync.dma_start(out=cA[:], in_=ca_in[:])
            xkb = cpool.tile([P, KC], f32, tag="c_xkb")
            nc.sync.dma_start(out=xkb[:], in_=xkb_in[:])
            xp = cpool.tile([P, M], f32, tag="c_x")
            nc.sync.dma_start(out=xp[:], in_=x_in[:])
            w2 = cpool.tile([P, P * NG], bf16, tag="c_w2")
            nc.scalar.dma_start(out=w2[:], in_=w2_in[:])
            cB = cpool.tile([NSLOT, 2], f32, tag="c_cb")
            nc.scalar.dma_start(out=cB[:], in_=cb_in[:])
            cM = cpool.tile([NSLOT, KC], bf16, tag="c_cm")
            nc.scalar.dma_start(out=cM[:], in_=cm_in[:])
            cW = cpool.tile([P, 40], bf16, tag="c_cw")
            nc.scalar.dma_start(out=cW[:], in_=cw_in[:])

            xleft = cA[:, 0:1]
            sc1 = cA[:, 1:4]
            bi1 = cA[:, 4:7]
            b2c = cA[:, 7:10]
            negl = cB[:, 0:1]
            rr = cB[:, 1:2]
            wmaskt = cM[:, 0:KC]
            w3f = cW[:, 0:36]
            one1 = cW[0:1, 36:37]
            ones12 = cW[0:NSLOT, 37:38]
            b3 = cW[0:NSLOT, 38:39]

            # ---- phase B: run the MLPs at the knots, mask windows ----
            h1s = []
            for g in range(NG):
                h1 = wpool.tile([P, KC], bf16, tag=f"h1_{g}")
                nc.scalar.activation(out=h1[:], in_=xkb[:], func=Act.Tanh,
                                     bias=bi1[:, g : g + 1],
                                     scale=sc1[:, g : g + 1])
                h1s.append(h1)
            s1 = wpool.tile([NSLOT, KC], f32, tag="s1")
            nc.scalar.activation(out=s1[:], in_=xkb[0:NSLOT, :],
                                 func=Act.Sigmoid, bias=negl, scale=1.0)
            s2 = wpool.tile([NSLOT, KC], f32, tag="s2")
            nc.scalar.activation(out=s2[:], in_=xkb[0:NSLOT, :],
                                 func=Act.Sigmoid, bias=rr, scale=-1.0)
            winm = wpool.tile([NSLOT, KC], bf16, tag="win")
            nc.vector.tensor_mul(out=winm[:], in0=s1[:], in1=s2[:])
            nc.vector.tensor_mul(out=winm[:], in0=winm[:], in1=wmaskt)

            pre = psum.tile([NSLOT, KC], f32, tag="pre")
            for g in range(NG):
                h2p = psum2.tile([P, KC], f32, tag="h2p")
                nc.tensor.matmul(out=h2p[:],
                                 lhsT=w2[:, g * P : (g + 1) * P],
                                 rhs=h1s[g][:], start=True, stop=True)
                h2 = wpool.tile([P, KC], bf16, tag="h2")
                nc.scalar.activation(out=h2[:], in_=h2p[:], func=Act.Tanh,
                                     bias=b2c[:, g : g + 1], scale=1.0)
                nc.tensor.matmul(out=pre[:],
                                 lhsT=w3f[:, g * NSLOT : (g + 1) * NSLOT],
                                 rhs=h2[:], start=(g == 0), stop=(g == NG - 1))

            # termw = pre*winm ; v = ones12^T @ termw + b3^T @ winm
            termw = wpool.tile([NSLOT, KC], bf16, tag="term")
            nc.vector.tensor_mul(out=termw[:], in0=pre[:], in1=winm[:])
            v_ps = psum.tile([1, KC], f32, tag="vps")
            nc.tensor.matmul(out=v_ps[:], lhsT=ones12, rhs=termw[:],
                             start=True, stop=False)
            nc.tensor.matmul(out=v_ps[:], lhsT=b3, rhs=winm[:],
                             start=False, stop=True)
            v_sb = wpool.tile([1, KC], bf16, tag="vsb")
            nc.vector.tensor_copy(out=v_sb[:], in_=v_ps[:])

            # ---- phase C: per-partition records via PE "transpose" ----
            vrec_ps = psum.tile([P, 2], f32, tag="vrec")
            nc.tensor.matmul(out=vrec_ps[:, 0:1], lhsT=v_sb[0:1, 0:P],
                             rhs=one1, start=True, stop=True)
            nc.tensor.matmul(out=vrec_ps[:, 1:2], lhsT=v_sb[0:1, 1 : P + 1],
                             rhs=one1, start=True, stop=True)
            vrec = wpool.tile([P, 2], f32, tag="vrecs")
            nc.vector.tensor_copy(out=vrec[:], in_=vrec_ps[:])
            dvc = wpool.tile([P, 1], f32, tag="dvc")
            nc.vector.tensor_sub(out=dvc[:], in0=vrec[:, 1:2], in1=vrec[:, 0:1])

            # ---- phase D: two tensor_scalar ops per point chunk ----
            for ch in range(CHUNKS):
                sl = slice(ch * MC, (ch + 1) * MC)
                t = ppool.tile([P, MC], f32, tag="t")
                nc.vector.tensor_scalar(out=t[:], in0=xp[:, sl], scalar1=xleft,
                                        scalar2=float(INVH), op0=Op.subtract,
                                        op1=Op.mult)
                y = ppool.tile([P, MC], f32, tag="y")
                nc.vector.tensor_scalar(out=y[:], in0=t[:], scalar1=dvc[:],
                                        scalar2=vrec[:, 0:1], op0=Op.mult,
                                        op1=Op.add)
                eng = nc.sync if ch % 2 == 0 else nc.scalar
                eng.dma_start(out=y_out[:, sl], in_=y[:])

    nc.compile()
    _PROGS[M] = nc
    return nc


# ---------------- host-side input prep ----------------------------------------
def _fold_weights(core, W1, b1, W2, b2, W3, b3):
    means, std, mid, Lb, Rb = _geometry()
    base = DOM0 + core * DW
    act = [w for w in range(NW) if (Rb[w] > base) and (Lb[w] < base + DW)]
    assert len(act) <= NSLOT, f"core {core}: {len(act)} active windows"
    sc1 = np.zeros((P, NG), np.float32)
    bi1 = np.zeros((P, NG), np.float32)
    w2blk = np.zeros((P, P * NG), np.float32)
    w3f = np.zeros((P, NSLOT * NG), np.float32)
    b2c = np.zeros((P, NG), np.float32)
    b3c = np.zeros((NSLOT, 1), np.float32)
    negl = np.zeros((NSLOT, 1), np.float32)
    rr = np.zeros((NSLOT, 1), np.float32)
    for slot, w in enumerate(act):
        g, s = divmod(slot, 4)
        rows = slice(32 * s, 32 * s + 32)
        w1r = W1[w, 0, :].astype(np.float64)
        sc1[rows, g] = (w1r / std[w]).astype(np.float32)
        bi1[rows, g] = (b1[w] - w1r * means[w] / std[w]).astype(np.float32)
        w2blk[rows, g * P + 32 * s : g * P + 32 * s + 32] = W2[w]
        w3f[rows, g * NSLOT + slot] = W3[w, :, 0]
        b2c[rows, g] = b2[w]
        b3c[slot, 0] = b3[w, 0]
        negl[slot, 0] = -mid[w]
        rr[slot, 0] = mid[w + 1]
    return sc1, bi1, w2blk, w3f, b2c, b3c, negl, rr, act


def _core_tables(core, act):
    """Knot x row and the masked tanh-folded window mask for one core."""
    means, std, mid, Lb, Rb = _geometry()
    base = np.float32(DOM0 + core * DW)
    xk = np.full(KC, np.float32(base + DW), np.float32)
    kidx = np.arange(P + 1, dtype=np.float64)
    xk[: P + 1] = (base + kidx * H).astype(np.float32)
    tanhk = np.tanh(xk.astype(np.float64)).astype(np.float32)
    wmaskt = np.zeros((NSLOT, KC), np.float32)
    for slot, w in enumerate(act):
        lbv = np.nextafter(Lb[w], -np.inf)
        m = (xk > lbv) & (xk < Rb[w])
        wmaskt[slot] = m.astype(np.float32) * tanhk
    return xk, wmaskt


def _prep_in_maps(inputs, M):
    x = np.asarray(inputs["x"], np.float32)
    W1 = np.asarray(inputs["W1"], np.float32)
    b1 = np.asarray(inputs["b1"], np.float32)
    W2 = np.asarray(inputs["W2"], np.float32)
    b2 = np.asarray(inputs["b2"], np.float32)
    W3 = np.asarray(inputs["W3"], np.float32)
    b3 = np.asarray(inputs["b3"], np.float32)

    # global cell of each point, stable sort, rank within cell
    cglob = np.minimum((x.astype(np.float64) * (1.0 / H)).astype(np.int64),
                       NCELL - 1)
    cglob = np.maximum(cglob, 0)
    order = np.argsort(cglob, kind="stable")
    cs = cglob[order]
    cnt = np.bincount(cglob, minlength=NCELL)
    maxcnt = int(cnt.max())
    if maxcnt > M:
        raise OverflowError(maxcnt)
    starts = np.concatenate(([0], np.cumsum(cnt)))
    rank = np.arange(len(x)) - starts[cs]
    slot = cs * M + rank                      # flat index into [NCELL, M]

    in_maps = []
    for core in range(NCORES):
        base = np.float32(DOM0 + core * DW)
        sc1, bi1, w2blk, w3f, b2c, b3c, negl, rr, act = _fold_weights(
            core, W1, b1, W2, b2, W3, b3)
        xk, wmaskt = _core_tables(core, act)
        xleft = xk[:P].astype(np.float32)
        # pad slots: cell's left edge -> t=0 -> y=vlo (finite, discarded)
        xpad = np.repeat(xleft, M)
        msk = (cs >= core * P) & (cs < (core + 1) * P)
        xpad[slot[msk] - core * P * M] = x[order[msk]]
        import ml_dtypes
        bf = ml_dtypes.bfloat16
        cA = np.zeros((P, 12), np.float32)
        cA[:, 0] = xleft
        cA[:, 1:4] = sc1
        cA[:, 4:7] = bi1
        cA[:, 7:10] = b2c
        cB = np.zeros((NSLOT, 2), np.float32)
        cB[:, 0:1] = negl
        cB[:, 1:2] = rr
        cW = np.zeros((P, 40), np.float32)
        cW[:, 0:36] = w3f
        cW[0, 36] = 1.0
        cW[0:NSLOT, 37] = 1.0
        cW[0:NSLOT, 38:39] = b3c
        in_maps.append({
            "x_pts": xpad.reshape(P, M),
            "cA": cA,
            "cB": cB,
            "cM": wmaskt.astype(bf),
            "cW": cW.astype(bf),
            "xkb": np.broadcast_to(xk, (P, KC)).copy(),
            "w2blk": w2blk.astype(bf),
        })
    return in_maps, order, slot


def _unpack(results, order, slot, n_total):
    allys = np.concatenate([r["y_out"].reshape(-1) for r in results])
    out = np.empty(n_total, np.float32)
    out[order] = allys[slot]
    return out


def kernel(**inputs) -> np.ndarray:
    from concourse.bass_utils import run_bass_kernel_spmd

    M = M_DEFAULT
    while True:
        try:
            in_maps, order, slot = _prep_in_maps(inputs, M)
            break
        except OverflowError as e:
            M = ((int(e.args[0]) + 31 + CHUNKS) // (4 * CHUNKS)) * 4 * CHUNKS
    nc = _build_program(M)
    res = run_bass_kernel_spmd(nc, in_maps, list(range(NCORES)))
    return _unpack(res.results, order, slot, len(np.asarray(inputs["x"])))


S_DEFAULT = M_DEFAULT  # test.py compat
